# revision 9
# baseline (speedup 1.0000x reference)
"""DeepAR Trainium2 Bass kernel, v3: fp8 DoubleRow + PE-side transposes.

Sharding: pure data parallel, batch 1024 -> 128 per core across 8 cores.

v3 over v2: the h->hT transpose moves from the DMA xbar to the PE array.
A DMA transpose costs ~2.2us of fixed latency on the recurrence path
(HWDGE 625ns + DGE-start 650ns + DMA sem propagation 900ns); a PE
transpose is 4x128-col matmul-transposes (~0.4us) already ordered inside
the PE FIFO, plus one DVE copy out of PSUM. To make PSUM room (16KB/
partition total), gate tensors live in per-bank [128,512] tiles from a
6-buffer pool (12KB) and transposes stage through a 2-buffer [128,1024]
bf16 pool (4KB).

Precision config (HW-verified): recurrent encoder matmuls h0/h1 in fp8
DoubleRow; L1 input matmul mixed (big-|W|-norm hidden units bf16, small
half fp8 DoubleRow, via a host-side hidden permutation); decoder bf16.
All gate contributions carry x256; fp8 operands carry x16 each side.
"""
import numpy as np
import ml_dtypes

import concourse.bass as bass
import concourse.mybir as mybir
import concourse.tile as tile
from concourse import bacc
from concourse.bass_utils import run_bass_kernel_spmd
from concourse.masks import make_identity

F32 = mybir.dt.float32
BF16 = mybir.dt.bfloat16
F8 = mybir.dt.float8e4
F8NP = ml_dtypes.float8_e4m3fn
AF = mybir.ActivationFunctionType
ALU = mybir.AluOpType
DR = mybir.MatmulPerfMode.DoubleRow

B, T_ENC, H_DEC = 1024, 168, 24
ENC_IN, DEC_IN, HID = 32, 16, 512
G = 4 * HID  # 2048
NS = 4       # psum banks (gate quarters) per gate tensor
NCORES = 8
BL = B // NCORES
XCHUNK = 28  # re-swept: 14/56 tested equal-or-worse

SCALE = 256.0
HSC = 16.0
FP8 = frozenset({"h0", "h1", "i1x"})
# Cell column-splitting: once the head ops were quarter-sliced, the split's
# extra ACT per-op overhead (~0.9us/cell) outweighs its chain-latency win
# everywhere; all cells run monolithic gate ops (sim: 1.603 vs 1.642ms).
CELL0_SPLIT = False
CELL1_SPLIT = False
CELLD_SPLIT = False
SPLIT_AT = 384  # cell half boundary (bank-quarter aligned: 128/256/384)
H1_VIA = "pe"   # h1's transpose: slack-rich chain stays off the DMA path
REC_KB_OUTER = True
SMP_BUFS = 3
HP_BUFS = 2
HTP_BUFS = 3
GB_BUFS = 6    # gate-bank psum tiles (2KB each); GB_BUFS*2 + PPT_BUFS*2 <= 16KB
PPT_BUFS = 2   # transpose-staging psum tiles (2KB each)

_PERM = np.concatenate([np.arange(1024, 1536), np.arange(0, 512),
                        np.arange(512, 1024), np.arange(1536, 2048)])


def _gate_perm(unit_order):
    blocks = [2, 0, 1, 3]  # [g,i,f,o] <- torch [i,f,g,o]
    return np.concatenate([b * 512 + unit_order for b in blocks])


def _bf16(x):
    return np.ascontiguousarray(x.astype(ml_dtypes.bfloat16))


def _f32(x):
    return np.ascontiguousarray(x.astype(np.float32))


def _wT_kxn_p(W, perm, scale=SCALE):
    Wt = W[perm].T * scale  # [D, 2048]
    D = Wt.shape[0]
    return _bf16(Wt.reshape(D // 128, 128, G).transpose(1, 0, 2))


def _wT_fp8_p(W, perm):
    Wt = W[perm].T * HSC  # [512, 2048]
    return np.ascontiguousarray(
        Wt.reshape(2, 2, 128, G).transpose(2, 0, 1, 3).astype(F8NP))


def _wT_kxn(W, scale=SCALE):
    return _wT_kxn_p(W, _PERM, scale)


def _wT_fp8(W):
    return _wT_fp8_p(W, _PERM)


def build_kernel(T=T_ENC, HD=H_DEC, fp8=FP8):
    nc = bacc.Bacc("TRN2", target_bir_lowering=False, debug=False,
                   num_devices=NCORES)

    def din(name, shape, dt):
        return nc.dram_tensor(name, shape, dt, kind="ExternalInput").ap()

    def wshape(key):
        return ([128, 2, 2, G], F8) if key in fp8 else ([128, 4, G], BF16)

    x_d = din("x", [ENC_IN + 1, T, BL], BF16)
    w0_d = din("w0", [ENC_IN + 1, G], BF16)
    wh0_d = din("wh0", *wshape("h0"))
    if "i1x" in fp8:
        wi1_d = (din("wi1b", [128, 2, G], BF16),
                 din("wi18", [128, 2, G], F8))
    else:
        wi1_d = din("wi1", *wshape("i1"))
    wh1_d = din("wh1", *wshape("h1"))
    whd_d = din("whd", *wshape("hd"))
    wctx_d = din("wctx", [128, 4, G], BF16)
    be_d = din("be", [33, G + 128], BF16)
    b1x_d = din("b1x", [65, G + 128], BF16)
    covy_d = din("covy", [DEC_IN + 1, HD, BL], BF16)
    wcy_d = din("wcy", [DEC_IN + 1, G], BF16)
    wms_d = din("wms", [128, 2 * HID + 2], F32)

    mu_d = nc.dram_tensor("mu", [BL, HD], F32, kind="ExternalOutput").ap()
    sg_d = nc.dram_tensor("sg", [BL, HD], F32, kind="ExternalOutput").ap()

    with tile.TileContext(nc) as tc:
        _emit(tc, T, HD, fp8, x_d, w0_d, wh0_d, wi1_d, wh1_d, whd_d, wctx_d,
              be_d, b1x_d, covy_d, wcy_d, wms_d, mu_d, sg_d)
    nc.compile()
    return nc


def _emit(tc, T, HD, fp8, x_d, w0_d, wh0_d, wi1_d, wh1_d, whd_d, wctx_d,
          be_d, b1x_d, covy_d, wcy_d, wms_d, mu_d, sg_d):
    nc = tc.nc
    mm = nc.tensor.matmul

    with (
        tc.tile_pool(name="const", bufs=1) as cp,
        tc.tile_pool(name="xp", bufs=2) as xp,
        tc.tile_pool(name="small", bufs=SMP_BUFS) as smp,
        tc.tile_pool(name="hp", bufs=HP_BUFS) as hp,
        tc.tile_pool(name="htp", bufs=HTP_BUFS) as htp,
        tc.tile_pool(name="ht8p", bufs=HTP_BUFS) as ht8p,
        tc.tile_pool(name="psum", bufs=GB_BUFS, space="PSUM") as pp,
        tc.tile_pool(name="psumT", bufs=PPT_BUFS, space="PSUM") as ppt,
    ):
        def load(name, dram, shape, dt):
            t = cp.tile(shape, dt, tag=name)
            nc.sync.dma_start(t[:], dram[:])
            return t

        w0 = load("w0", w0_d, [ENC_IN + 1, G], BF16)
        wh0 = load("wh0", wh0_d,
                   [128, 2, 2, G] if "h0" in fp8 else [128, 4, G],
                   F8 if "h0" in fp8 else BF16)
        be = load("be", be_d, [33, G + 128], BF16)
        b1x = load("b1x", b1x_d, [65, G + 128], BF16)
        if "i1x" in fp8:
            wi1 = (load("wi1b", wi1_d[0], [128, 2, G], BF16),
                   load("wi18", wi1_d[1], [128, 2, G], F8))
        elif "i1" in fp8:
            wi1 = load("wi1", wi1_d, [128, 2, 2, G], F8)
        else:
            wi1 = load("wi1", wi1_d, [128, 4, G], BF16)
        wh1 = load("wh1", wh1_d,
                   [128, 2, 2, G] if "h1" in fp8 else [128, 4, G],
                   F8 if "h1" in fp8 else BF16)
        dec_w = {}

        def load_decoder_weights():
            # emitted a few iterations in: the DMA queue drains the encoder's
            # x chunk + recurrent weights first, but these still land ~1.4ms
            # before the decoder needs them
            dec_w["wctx"] = load("wctx", wctx_d, [128, 4, G], BF16)
            dec_w["whd"] = load(
                "whd", whd_d,
                [128, 2, 2, G] if "hd" in fp8 else [128, 4, G],
                F8 if "hd" in fp8 else BF16)
            dec_w["covy"] = load("covy", covy_d, [DEC_IN + 1, HD, BL], BF16)
            dec_w["wcy"] = load("wcy", wcy_d, [DEC_IN + 1, G], BF16)
            dec_w["wms"] = load("wms", wms_d, [128, 2 * HID + 2], F32)

        ident = cp.tile([128, 128], BF16, tag="ident")
        make_identity(nc, ident[:])

        ones32_r = be[32:33, G:G + 128]
        bd_r = be[32:33, 0:G]
        # L1 bias lives at partition 64 (PE row group 2): its matmuls run
        # concurrently with the x matmuls (row groups 0-1) on HW
        ones64_r = b1x[64:65, G:G + 128]
        b1_r = b1x[64:65, 0:G]

        c0 = cp.tile([128, HID], BF16, tag="c0")
        c1 = cp.tile([128, HID], BF16, tag="c1")
        cd = cp.tile([128, HID], BF16, tag="cd")
        mu_b = cp.tile([128, HD], F32, tag="mu_b")
        sp_b = cp.tile([128, HD], F32, tag="sp_b")
        sg_b = cp.tile([128, HD], F32, tag="sg_b")
        mu_q = cp.tile([128, 4 * HD], F32, tag="mu_q")
        sp_q = cp.tile([128, 4 * HD], F32, tag="sp_q")

        inv = 1.0 / SCALE

        def gtile():
            """gate tensor = 4 per-bank [128,512] psum tiles [g|i|f|o]."""
            return [pp.tile([128, 512], F32, tag="gb", name=f"gb{n}")
                    for n in range(NS)]

        def cell(g, c, first, h_tag, ve, split=True):
            """per-bank gates [g|i|f|o] x256 -> h bf16, optionally split.

            The split is asymmetric (SPLIT_AT): the recurrence waits on the
            LAST half's chain (its DR block converts last), so a small late
            half lands h's tail sooner at the same total ACT cost.
            """
            H2 = SPLIT_AT if split else HID
            tg = smp.tile([128, HID], BF16, tag="tg")
            si = smp.tile([128, HID], BF16, tag="si")
            sf = smp.tile([128, HID], BF16, tag="sf")
            so = smp.tile([128, HID], BF16, tag="so")
            m1 = smp.tile([128, HID], BF16, tag="m1")
            m2 = smp.tile([128, HID], BF16, tag="m2")
            tcn = smp.tile([128, HID], BF16, tag="tc")
            h = hp.tile([128, HID], BF16, tag=h_tag)
            halves = ([slice(0, H2), slice(H2, HID)] if split
                      else [slice(0, HID)])
            for s in halves:
                nc.scalar.activation(tg[:, s], g[0][:, s], AF.Tanh, scale=inv)
                nc.scalar.activation(si[:, s], g[1][:, s], AF.Sigmoid,
                                     scale=inv)
                nc.scalar.activation(sf[:, s], g[2][:, s], AF.Sigmoid,
                                     scale=inv)
                nc.scalar.activation(so[:, s], g[3][:, s], AF.Sigmoid,
                                     scale=inv)
            for s in halves:
                if first:
                    ve.tensor_mul(c[:, s], si[:, s], tg[:, s])
                else:
                    ve.tensor_mul(m1[:, s], si[:, s], tg[:, s])
                    ve.tensor_mul(m2[:, s], sf[:, s], c[:, s])
                    ve.tensor_add(c[:, s], m1[:, s], m2[:, s])
            for s in halves:
                nc.scalar.activation(tcn[:, s], c[:, s], AF.Tanh)
                ve.tensor_mul(h[:, s], so[:, s], tcn[:, s])
            return h

        def transp(h, tag, need_bf, need_f8, ve, bf_planes=4, via="pe"):
            """h [128,512] -> hT.

            via="pe": 4 matmul-transposes through PSUM (~0.4us of PE, no DMA
            fixed latency) -- for chains on the recurrence critical path.
            via="dma": one DmaTransposeAnt into SBUF (~2.2us latency: HWDGE
            + DGE start + DMA sem propagation) but zero PE cost -- for
            chains with >=1 iteration of slack (h1). The fp8 convert runs
            per DoubleRow block so kb0 matmuls start off the first half.
            """
            out = {"bf": None, "f8": None}
            if via == "dma":
                ht = htp.tile([128, 4, 128], BF16, tag=tag)
                nc.sync.dma_start(ht[:], h[:], transpose=True)
                out["bf"] = ht
                if need_f8:
                    h8 = ht8p.tile([128, 4, 128], F8, tag=tag + "8")
                    ve.tensor_scalar_mul(h8[:, 0:2, :], ht[:, 0:2, :], HSC)
                    ve.tensor_scalar_mul(h8[:, 2:4, :], ht[:, 2:4, :], HSC)
                    out["f8"] = h8
                return out
            ps = ppt.tile([128, 1024], BF16, tag="hTps")
            for k in range(4):
                mm(ps[:, k * 128:(k + 1) * 128], h[:, k * 128:(k + 1) * 128],
                   ident[:], is_transpose=True, start=(k == 0), stop=(k == 3))
            if need_f8:
                h8 = ht8p.tile([128, 4, 128], F8, tag=tag + "8")
                ve.tensor_scalar_mul(h8[:, 0:2, :], ps[:, 0:256], HSC)
                ve.tensor_scalar_mul(h8[:, 2:4, :], ps[:, 256:512], HSC)
                out["f8"] = h8
            if need_bf:
                ht = htp.tile([128, 4, 128], BF16, tag=tag)
                ve.tensor_copy(ht[:, 0:2, :], ps[:, 0:256])
                if bf_planes > 2:
                    ve.tensor_copy(ht[:, 2:4, :], ps[:, 256:512])
                out["bf"] = ht
            return out

        def rec_mm(g, hT, w, key, start, stop, kb_only=None):
            """hT @ w accumulated into per-bank g tiles."""
            if key == "i1" and "i1x" in fp8:
                wb, w8 = w
                hb, h8 = hT["bf"], hT["f8"]
                for n in range(NS):
                    s = slice(n * 512, (n + 1) * 512)
                    for k in range(2):
                        mm(g[n][:], hb[:, k, :], wb[:, k, s],
                           start=start and k == 0, stop=False)
                    mm(g[n][:], h8[:, 2:4, :], w8[:, :, s],
                       start=False, stop=stop, perf_mode=DR)
                return
            if key in fp8:
                h8 = hT["f8"]
                kbs = range(2) if kb_only is None else [kb_only]
                for kb in kbs:
                    for n in range(NS):
                        s = slice(n * 512, (n + 1) * 512)
                        mm(g[n][:], h8[:, 2 * kb:2 * kb + 2, :],
                           w[:, kb, :, s], start=start and kb == 0,
                           stop=stop and kb == 1, perf_mode=DR)
            else:
                hb = hT["bf"]
                for n in range(NS):
                    s = slice(n * 512, (n + 1) * 512)
                    for k in range(4):
                        mm(g[n][:], hb[:, k, :], w[:, k, s],
                           start=start and k == 0, stop=stop and k == 3)

        # ================= encoder =================
        h0T_hist = {}
        h1T = None
        x_sb = None
        need_h0bf = ("i1x" in fp8) or ("i1" not in fp8)

        def xbias_mm(g0, g1, t, ti):
            for n in range(NS):
                s = slice(n * 512, (n + 1) * 512)
                mm(g0[n][:], x_sb[:, ti, :], w0[:, s],
                   start=True, stop=(t == 0))
                if g1 is not None:
                    mm(g1[n][:], ones64_r, b1_r[:, s], start=True, stop=False,
                       tile_position=(64, 0))

        for t in range(T):
            if t % XCHUNK == 0:
                nx = min(XCHUNK, T - t)
                x_sb = xp.tile([ENC_IN + 1, XCHUNK, BL], BF16, tag="x")
                nc.sync.dma_start(x_sb[:, :nx, :], x_d[:, t:t + nx, :])
            ti = t % XCHUNK
            if t == 2:
                load_decoder_weights()
            g0 = gtile()
            g1 = gtile() if t >= 1 else None
            xbias_mm(g0, g1, t, ti)
            if t > 0:
                rec_mm(g0, h0T_hist[t - 1], wh0, "h0", False, True)
                rec_mm(g1, h0T_hist[t - 1], wi1, "i1", False, t == 1)
                if t > 1:
                    rec_mm(g1, h1T, wh1, "h1", False, True)
            h0 = cell(g0, c0, t == 0, "h0", nc.vector, split=CELL0_SPLIT)
            h0T_hist[t] = transp(h0, "h0T", need_h0bf, "h0" in fp8,
                                 nc.vector,
                                 bf_planes=2 if "i1x" in fp8 else 4)
            h0T_hist.pop(t - 2, None)
            if t > 0:
                h1 = cell(g1, c1, t == 1, "h1", nc.vector, split=CELL1_SPLIT)
                h1T = transp(h1, "h1T", False, "h1" in fp8, nc.vector,
                             via=H1_VIA)
        # drain: L1's last step
        g1 = gtile()
        for n in range(NS):
            s = slice(n * 512, (n + 1) * 512)
            mm(g1[n][:], ones64_r, b1_r[:, s], start=True, stop=False,
               tile_position=(64, 0))
        rec_mm(g1, h0T_hist[T - 1], wi1, "i1", False, False)
        rec_mm(g1, h1T, wh1, "h1", False, True)
        h1 = cell(g1, c1, False, "h1", nc.vector, split=CELL1_SPLIT)
        h1T = transp(h1, "h1T", True, False, nc.vector)

        # ================= decoder =================
        # ctx_pre = (context @ W_ctx.T + b_d) x256, via bf16 h1T
        wctx, whd = dec_w["wctx"], dec_w["whd"]
        covy, wcy, wms = dec_w["covy"], dec_w["wcy"], dec_w["wms"]
        cps = gtile()
        for n in range(NS):
            s = slice(n * 512, (n + 1) * 512)
            mm(cps[n][:], ones32_r, bd_r[:, s], start=True, stop=False)
        for k in range(4):
            for n in range(NS):
                s = slice(n * 512, (n + 1) * 512)
                mm(cps[n][:], h1T["bf"][:, k, :], wctx[:, k, s],
                   start=False, stop=(k == 3))
        ctxp = cp.tile([128, G], BF16, tag="ctxp")
        for n in range(NS):
            nc.scalar.copy(ctxp[:, n * 512:(n + 1) * 512], cps[n][:])

        hdT = None
        for t in range(HD):
            gd = gtile()
            for n in range(NS):
                s = slice(n * 512, (n + 1) * 512)
                mm(gd[n][:], ident[:], ctxp[:, s], start=True, stop=False)
                mm(gd[n][:], covy[:, t, :], wcy[:, s],
                   start=False, stop=(t == 0))
            if t > 0:
                rec_mm(gd, hdT, whd, "hd", False, True)
            hd = cell(gd, cd, t == 0, "hd", nc.vector, split=CELLD_SPLIT)
            hdT = transp(hd, "hdT", "hd" not in fp8, "hd" in fp8, nc.vector)

            # heads in quarter slices: a monolithic 594ns stt op parked on
            # the DVE queue delays the hdT copies (which gate step t+1's
            # rec matmuls) by up to 1.2us; 150ns slices yield quickly.
            # accum_out sums per-op, so quarters accumulate via tmp columns
            # summed at the end (see the mu_q/sp_q reduction below).
            Q = HID // 4
            hsc = smp.tile([128, HID], F32, tag="hsc")
            hsc2 = smp.tile([128, HID], F32, tag="hsc2")
            for q in range(4):
                sq = slice(q * Q, (q + 1) * Q)
                nc.vector.scalar_tensor_tensor(
                    hsc[:, sq], hd[:, sq], 1.0, wms[:, 0:HID][:, sq],
                    op0=ALU.mult, op1=ALU.mult,
                    accum_out=mu_q[:, 4 * t + q:4 * t + q + 1])
                nc.vector.scalar_tensor_tensor(
                    hsc2[:, sq], hd[:, sq], 1.0, wms[:, HID:2 * HID][:, sq],
                    op0=ALU.mult, op1=ALU.mult,
                    accum_out=sp_q[:, 4 * t + q:4 * t + q + 1])

        # sum the per-quarter head partials: [128, HD*4] -> [128, HD]
        mu_q3 = mu_q[:].rearrange("p (t q) -> p t q", q=4)
        sp_q3 = sp_q[:].rearrange("p (t q) -> p t q", q=4)
        nc.vector.tensor_add(mu_b[:], mu_q3[:, :, 0], mu_q3[:, :, 1])
        nc.vector.tensor_add(sg_b[:], mu_q3[:, :, 2], mu_q3[:, :, 3])
        nc.vector.tensor_add(mu_b[:], mu_b[:], sg_b[:])
        nc.vector.tensor_add(sp_b[:], sp_q3[:, :, 0], sp_q3[:, :, 1])
        nc.vector.tensor_add(sg_b[:], sp_q3[:, :, 2], sp_q3[:, :, 3])
        nc.vector.tensor_add(sp_b[:], sp_b[:], sg_b[:])
        nc.vector.tensor_scalar_add(mu_b[:], mu_b[:],
                                    wms[:, 2 * HID:2 * HID + 1])
        nc.vector.tensor_scalar_add(sp_b[:], sp_b[:],
                                    wms[:, 2 * HID + 1:2 * HID + 2])
        nc.scalar.activation(sp_b[:], sp_b[:], AF.Exp)
        nc.scalar.activation(sg_b[:], sp_b[:], AF.Ln, bias=1.0)
        nc.vector.tensor_scalar_add(sg_b[:], sg_b[:], 1e-6)
        nc.sync.dma_start(mu_d[:], mu_b[:])
        nc.sync.dma_start(sg_d[:], sg_b[:])


def _wi1_maps(W_ih1, fp8):
    if "i1x" in fp8:
        Wt = W_ih1[_PERM].T  # [512, 2048], rows = h0 units (permuted)
        wb = _bf16((Wt[:256] * SCALE).reshape(2, 128, G).transpose(1, 0, 2))
        w8 = np.ascontiguousarray(
            (Wt[256:] * HSC).reshape(2, 128, G).transpose(1, 0, 2)
            .astype(F8NP))
        return {"wi1b": wb, "wi18": w8}
    if "i1" in fp8:
        return {"wi1": _wT_fp8(W_ih1)}
    return {"wi1": _wT_kxn(W_ih1)}


def _make_be(bdv):
    be = np.zeros((33, G + 128), np.float32)
    be[32, :G] = bdv * SCALE
    be[32, G:] = 1.0
    return _bf16(be)


def _make_b1x(b1):
    bx = np.zeros((65, G + 128), np.float32)
    bx[64, :G] = b1 * SCALE
    bx[64, G:] = 1.0
    return _bf16(bx)


def _make_wms(W_mu, W_sig, b_mu, b_sig):
    w = np.zeros((128, 2 * HID + 2), np.float32)
    w[:, 0:HID] = W_mu[0][None, :]
    w[:, HID:2 * HID] = W_sig[0][None, :]
    w[:, 2 * HID] = b_mu[0]
    w[:, 2 * HID + 1] = b_sig[0]
    return _f32(w)


def prep_inputs(inputs, T=T_ENC, HD=H_DEC, fp8=FP8):
    enc = _f32(np.asarray(inputs["enc_inp"]))[:, :T]
    dec = _f32(np.asarray(inputs["dec_inp"]))[:, :HD]
    tgt = _f32(np.asarray(inputs["tgt"]))[:, :HD]

    W_ih0, W_hh0 = np.asarray(inputs["W_ih0"]), np.asarray(inputs["W_hh0"])
    W_ih1, W_hh1 = np.asarray(inputs["W_ih1"]), np.asarray(inputs["W_hh1"])
    W_ihd, W_hhd = np.asarray(inputs["W_ihd"]), np.asarray(inputs["W_hhd"])
    if "i1x" in fp8:
        order = np.argsort(-np.linalg.norm(W_ih1, axis=0))
        p0 = _gate_perm(order)
        W_hh0 = W_hh0[:, order]
        W_ih1 = W_ih1[:, order]
    else:
        p0 = _PERM
    b0 = _f32(np.asarray(inputs["b_ih0"]) + np.asarray(inputs["b_hh0"]))[p0]
    b1 = _f32(np.asarray(inputs["b_ih1"]) + np.asarray(inputs["b_hh1"]))[_PERM]
    bdv = _f32(np.asarray(inputs["b_ihd"]) + np.asarray(inputs["b_hhd"]))[_PERM]
    W_mu, b_mu = np.asarray(inputs["W_mu"]), np.asarray(inputs["b_mu"])
    W_sig, b_sig = np.asarray(inputs["W_sig"]), np.asarray(inputs["b_sig"])

    def wrec(W, key):
        return _wT_fp8(W) if key in fp8 else _wT_kxn(W)

    w0 = np.concatenate([W_ih0[p0].T * SCALE, SCALE * b0[None, :]], 0)
    shared = {
        "w0": _bf16(w0),
        "wh0": _wT_fp8_p(W_hh0, p0) if "h0" in fp8 else _wT_kxn_p(W_hh0, p0),
        "wh1": wrec(W_hh1, "h1"),
        "whd": wrec(W_hhd, "hd"),
        **_wi1_maps(W_ih1, fp8),
        "wctx": _wT_kxn(W_ihd[:, DEC_IN:DEC_IN + HID]),
        "be": _make_be(bdv),
        "b1x": _make_b1x(b1),
        "wcy": _bf16(SCALE * np.concatenate(
            [W_ihd[_PERM][:, :DEC_IN].T, W_ihd[_PERM][:, DEC_IN + HID:].T], 0)),
        "wms": _make_wms(W_mu, W_sig, b_mu, b_sig),
    }

    in_maps = []
    for c in range(NCORES):
        sl = slice(c * BL, (c + 1) * BL)
        xe = np.ones((ENC_IN + 1, T, BL), np.float32)
        xe[:ENC_IN] = enc[sl].transpose(2, 1, 0)
        cy = np.zeros((DEC_IN + 1, HD, BL), np.float32)
        cy[:DEC_IN] = dec[sl].transpose(2, 1, 0)
        cy[DEC_IN, 1:] = tgt[sl, :HD - 1].T
        m = dict(shared)
        m["x"] = _bf16(xe)
        m["covy"] = _bf16(cy)
        in_maps.append(m)
    return in_maps


_NC_CACHE = {}


def _get_nc(T=T_ENC, HD=H_DEC):
    key = (T, HD)
    if key not in _NC_CACHE:
        _NC_CACHE[key] = build_kernel(T, HD)
    return _NC_CACHE[key]


def run(inputs, T=T_ENC, HD=H_DEC, **kw):
    nc = _get_nc(T, HD)
    in_maps = prep_inputs(inputs, T, HD)
    res = run_bass_kernel_spmd(nc, in_maps, core_ids=list(range(NCORES)), **kw)
    mu = np.concatenate([res.results[c]["mu"] for c in range(NCORES)], 0)
    sg = np.concatenate([res.results[c]["sg"] for c in range(NCORES)], 0)
    return (mu, sg), res


def kernel(**inputs):
    (mu, sg), _ = run(inputs)
    return mu, sg
